# revision 31
# baseline (speedup 1.0000x reference)
"""Trainium2 Bass kernel for submanifold sparse conv net (gnn_message_passing).

Network: mask = (x != 0); y = BN(x) masked; y1 = relu(subm_conv3x3(y, w1) + b1);
y2 = relu(subm_conv3x3(y1, w2) + b2); out = NCHW(y2).  B,H,W = 4,512,512, C: 1->32->64.

Sharding: H split into 8 slabs of 64 rows (one per NeuronCore).

Per-core design ("quad" layout, channels*rows on partitions):

- conv1 computes FOUR y1 rows per matmul: PSUM [128 = 4 rows x 32 ch, 512].
  K=22 rhs rows are (row-shift s in 0..5) x (dw in -1..1) copies of the
  host-precombined plane P = bn_scale*x + bn_shift*mask (18 rows) plus 4
  shifted mask rows (submanifold restrict via +LARGE, bias -LARGE).
- y1 (bf16) is stored in two phase-interleaved banks of 4-row "slots" so
  every conv2 output row-pair (r, r+1) finds its full 4-row x 32-ch input
  window on 128 partitions in ONE slot: A-slots hold rows 4q-1..4q+2
  (pairs r=4t), B-slots hold rows 4w+1..4w+4 band-rotated (pairs r=4t+2).
  The B copies reuse the same conv1 PSUM partitions (no transpose).
- conv2: per row-pair, 3 bf16 matmuls [K=128, M=128 = 2 rows x 64 ch,
  N=512] (one per dw; dw = +-1 is a free-dim offset thanks to 1-col slot
  pads) plus ONE fp8 DoubleRow matmul adding +LARGE*mask for both rows
  (mask rows live in a [128, 516] fp8 tile, one partition per (band, quad)).
  Relu+bias+bf16-downcast rotates across ACT/DVE/GPSIMD.
- Output written bf16 (upcast on host): halves the dominant DMA traffic.
"""

import sys

if "/opt/trn_rl_repo" not in sys.path:
    sys.path.insert(0, "/opt/trn_rl_repo")

import numpy as np
import ml_dtypes

BF16 = ml_dtypes.bfloat16
F8 = ml_dtypes.float8_e4m3   # device float8e4 has inf: max finite 240

B, H, W = 4, 512, 512
NCORES = 8
ROWS = H // NCORES          # 64 output rows per core
WP = 514                    # padded row width (1 zero col each side)
NQ = 17                     # conv1 quads per batch (y1 rows -1..66)
PR = 70                     # P slab rows (-2..67)
MR = 68                     # mask slab rows (-1..66)
PMB = 22 * NQ * 512         # pm elems per batch (pre-gathered conv1 rhs)
MQB = MR * 512              # mq elems per batch
ASLOTS = 17
BSLOTS = 16
BBASE = ASLOTS * WP         # B-slot region base inside y1 tile
REG = (ASLOTS + BSLOTS) * WP
OUTB = 64 * ROWS * W        # out elems per batch per core
LARGE = 256.0
LARGEM = 224.0  # mask weight in fp8e4m3 (256 would be inf)
EPS = 1e-5

_cached = {}


def _build_nc():
    import concourse.bass as bass
    import concourse.mybir as mybir
    from concourse import bacc, tile

    f32 = mybir.dt.float32
    bf16 = mybir.dt.bfloat16
    fp8 = mybir.dt.float8e4
    AP = bass.AP
    DR = mybir.MatmulPerfMode.DoubleRow
    Relu = mybir.ActivationFunctionType.Relu
    add_, max_ = mybir.AluOpType.add, mybir.AluOpType.max

    nc = bacc.Bacc("TRN2", target_bir_lowering=False, debug=False,
                   num_devices=NCORES)
    pm = nc.declare_dram_parameter("pm", [B * PMB], bf16, isOutput=False)
    mqd = nc.declare_dram_parameter("mq", [B * MQB], fp8, isOutput=False)
    wbd = nc.declare_dram_parameter("wb", [128 * 896], bf16, isOutput=False)
    wmd = nc.declare_dram_parameter("wm", [128 * 8192], fp8, isOutput=False)
    bid = nc.declare_dram_parameter("bias", [128 * 2], f32, isOutput=False)
    out = nc.declare_dram_parameter("out", [B * OUTB], bf16, isOutput=True)

    with tile.TileContext(nc) as tc:
        with (
            tc.tile_pool(name="const", bufs=1) as cpool,
            tc.tile_pool(name="rhs1", bufs=2) as rpool,
            tc.tile_pool(name="y1", bufs=2) as ypool,
            tc.tile_pool(name="mq", bufs=4) as mpool,
            tc.tile_pool(name="stage", bufs=3) as spool,
            tc.tile_pool(name="ps1", bufs=2, space="PSUM") as p1pool,
            tc.tile_pool(name="ps2", bufs=2, space="PSUM") as p2pool,
        ):
            wb = cpool.tile([128, 896], bf16, tag="wb")
            # wm as 3D [128, 64, 128]: pair i = dim1 slots 2i, 2i+1, so the
            # DR lhsT [K, 2, M] is a plain (tracked) slice
            wm = cpool.tile([128, 64, 128], fp8, tag="wm")
            bias = cpool.tile([128, 2], f32, tag="bias")
            nc.sync.dma_start(out=wb[:, :], in_=AP(wbd, 0, [[896, 128], [1, 896]]))
            nc.sync.dma_start(out=bias[:, :], in_=AP(bid, 0, [[2, 128], [1, 2]]))
            w1t = wb[0:22, 0:128]
            WA = [wb[0:128, 128 + 128 * d:256 + 128 * d] for d in range(3)]
            WBm = [wb[0:128, 512 + 128 * d:640 + 128 * d] for d in range(3)]
            bias1 = bias[:, 0:1]
            bias2 = bias[:, 1:2]
            # warm engines against the const DMA lanes so first real ops
            # don't exceed the per-instruction sync-wait limit
            scr = cpool.tile([128, 2], f32, tag="scr")
            nc.scalar.activation(scr[:, 0:1], bias[:, 0:1], Relu, bias=bias[:, 1:2])
            nc.vector.tensor_scalar(scr[:, 1:2], bias[:, 1:2], 0.0, 0.0, add_, max_)

            # GPSIMD cannot touch PSUM: relus (PSUM reads) go to ACT/DVE;
            # y1 B-slot fills are SBUF->SBUF bf16 copies (DVE 4x mode).
            def relu_op(eng, out_ap, in_ap, bias_ap):
                if eng == 0:
                    nc.scalar.activation(out_ap, in_ap, Relu, bias=bias_ap)
                else:
                    nc.vector.tensor_scalar(out_ap, in_ap, bias_ap, 0.0, add_, max_)

            y2rot = {"i": 0}
            # per 16-group batch: 7 on ACT, 9 on DVE
            Y2PAT = (1, 0, 1, 1, 0, 1, 0, 1, 1, 0, 1, 0, 1, 1, 0, 1)

            def next_y2():
                e = Y2PAT[y2rot["i"] % 16]
                y2rot["i"] += 1
                return e

            for b in range(B):
                rhs1 = rpool.tile([22, NQ * 512], bf16, tag="rhs1")
                nc.sync.dma_start(
                    out=rhs1[:, :],
                    in_=AP(pm, b * PMB, [[NQ * 512, 22], [1, NQ * 512]]))
                if b == 0:
                    # issue the big wm load after batch 0's input so the
                    # first conv1 isn't stuck behind its transfer
                    nc.sync.dma_start(out=wm[:, :, :],
                                      in_=AP(wmd, 0, [[8192, 128], [1, 8192]]))
                # mq 3D [128, 2, 516]: copy 0 holds mask rows (partition
                # 4q+bq = mask row 4q-1+bq), copy 1 stays zero -- the DR
                # rhs [:, 0:2, 0:512] is then a plain tracked slice (the
                # second tile multiplies zero weights anyway).
                mq = mpool.tile([128, 2, 516], fp8, tag="mq")
                nc.gpsimd.memset(mq[:, :, :], 0.0)
                nc.sync.dma_start(
                    out=mq[0:MR, 0:1, 0:512],
                    in_=AP(mqd, b * MQB, [[512, MR], [1, 512]]))
                # y1 as a 3D tile [128, 33 slots, 514]: slots 0..16 = A (rows
                # 4q-1..4q+2), 17..32 = B slot w (rows 4w+1..4w+4, bands
                # rotated).  All accesses are plain slices so the tile
                # scheduler tracks them (raw APs are invisible to it).
                y1 = ypool.tile([128, ASLOTS + BSLOTS, WP], bf16, tag="y1")
                nc.gpsimd.memset(y1[:, :, 0:1], 0.0)
                nc.gpsimd.memset(y1[:, :, WP - 1:WP], 0.0)

                stage_tiles = {}

                def emit_group(t):
                    ps2 = p2pool.tile([128, 1024], f32, tag="ps2")
                    # aligned pair (rows 4t, 4t+1) from A-slot t
                    for d in range(3):
                        nc.tensor.matmul(
                            ps2[:, 0:512], lhsT=WA[d],
                            rhs=y1[:, t:t + 1, d:d + 512],
                            start=(d == 0), stop=False)
                    nc.tensor.matmul(
                        ps2[:, 0:512],
                        lhsT=wm[:, 2 * t:2 * t + 2, 0:128],
                        rhs=mq[:, 0:2, 0:512],
                        start=False, stop=True, perf_mode=DR)
                    # split pair (rows 4t+2, 4t+3) from B-slot t
                    for d in range(3):
                        nc.tensor.matmul(
                            ps2[:, 512:1024], lhsT=WBm[d],
                            rhs=y1[:, ASLOTS + t:ASLOTS + t + 1, d:d + 512],
                            start=(d == 0), stop=False)
                    nc.tensor.matmul(
                        ps2[:, 512:1024],
                        lhsT=wm[:, 32 + 2 * t:32 + 2 * t + 2, 0:128],
                        rhs=mq[:, 0:2, 0:512],
                        start=False, stop=True, perf_mode=DR)
                    s = t // 2
                    if t % 2 == 0:
                        stage_tiles[s] = spool.tile([128, 2048], bf16,
                                                    tag="stage", name="stage")
                    st = stage_tiles[s]
                    relu_op(next_y2(), st[:, (t % 2) * 1024:(t % 2) * 1024 + 1024],
                            ps2[:, :], bias2)
                    if t % 2 == 1:
                        # out DRAM mirrors the stage layout exactly
                        # [B][8 stages][128 part][4 pairsel][512]; host fixes
                        # the ordering.  One 2-dim AP, 4KB descriptors.
                        nc.sync.dma_start(
                            out=AP(out, (b * 8 + s) * 128 * 2048,
                                   [[2048, 128], [1, 2048]]),
                            in_=st[:, :])
                        del stage_tiles[s]

                BA = ASLOTS  # B-slot w lives at y1 slot BA + w
                for k in range(9):
                    ps1 = p1pool.tile([128, 1024], f32, tag="ps1")
                    nhalf = 2 if k < 8 else 1
                    for h in range(nhalf):
                        q = 2 * k + h
                        nc.tensor.matmul(
                            ps1[:, 512 * h:512 * h + 512], lhsT=w1t,
                            rhs=rhs1[:, q * 512:q * 512 + 512],
                            start=True, stop=True)
                    if k < 8:
                        # A-write: rows 4(2k)-1 .. 4(2k+1)+2 -> A-slots 2k, 2k+1
                        nc.scalar.activation(
                            y1[:, 2 * k:2 * k + 2, 1:513],
                            ps1[:, :], Relu, bias=bias1)
                        # B-slot fills are SBUF->SBUF copies of A data (same
                        # partitions): B-low = bands 2,3 of A 2k, 2k+1;
                        # B-high = bands 0,1 of A 2k, 2k+1 -> B 2k-1, 2k.
                        nc.vector.tensor_scalar(
                            y1[64:128, BA + 2 * k:BA + 2 * k + 2, 1:513],
                            y1[64:128, 2 * k:2 * k + 2, 1:513],
                            0.0, None, add_)
                        if k == 0:
                            nc.vector.tensor_scalar(
                                y1[0:64, BA:BA + 1, 1:513],
                                y1[0:64, 1:2, 1:513],
                                0.0, None, add_)
                        else:
                            nc.vector.tensor_scalar(
                                y1[0:64, BA + 2 * k - 1:BA + 2 * k + 1, 1:513],
                                y1[0:64, 2 * k:2 * k + 2, 1:513],
                                0.0, None, add_)
                    else:
                        # lone quad 16: only bands 0,1 (rows 63, 64) -> B-slot 15
                        relu_op(0,
                                y1[0:64, BA + 15:BA + 16, 1:513],
                                ps1[0:64, 0:512], bias1[0:64, 0:1])
                    # conv2 groups, one k-iteration BEHIND readiness: the
                    # in-order PE queue then never waits on the ACT->DVE
                    # y1-fill chain (which also protects the PE p-state).
                    if k == 2:
                        emit_group(0)
                    elif 3 <= k <= 8:
                        emit_group(2 * k - 5)
                        emit_group(2 * k - 4)
                for t in (13, 14, 15):
                    emit_group(t)
    nc.finalize()
    return nc


def _prep_consts(bn_gamma, bn_beta, bn_mean, bn_var, w1, b1, w2, b2):
    w1 = np.asarray(w1, np.float32)  # [3,3,1,32] (kh, kw, ci, co)
    w2 = np.asarray(w2, np.float32)  # [3,3,32,64]
    b1 = np.asarray(b1, np.float32)
    b2 = np.asarray(b2, np.float32)

    wb = np.zeros((128, 896), np.float32)
    # conv1 lhsT: rows kp = s*3+dwi (P copies), 18+b (mask rows); cols b*32+ch
    for s in range(6):
        for dwi in range(3):
            kp = s * 3 + dwi
            for bq in range(4):
                dh = s - bq
                if 0 <= dh <= 2:
                    wb[kp, 128 * 0:0] = 0  # noop for clarity
                    wb[kp, bq * 32:(bq + 1) * 32] = w1[dh, dwi, 0, :]
    for bq in range(4):
        wb[18 + bq, bq * 32:(bq + 1) * 32] = LARGE
    # conv2 aligned lhsT WA[d]: row (b*32+ch), col (par*64+co)
    for d in range(3):
        WAd = np.zeros((128, 128), np.float32)
        WBd = np.zeros((128, 128), np.float32)
        for bq in range(4):
            for par in range(2):
                dh = bq - par
                if 0 <= dh <= 2:
                    WAd[bq * 32:(bq + 1) * 32, par * 64:(par + 1) * 64] = w2[dh, d, :, :]
                dhB = (2 - par, 3 - par, -par, 1 - par)[bq]
                if 0 <= dhB <= 2:
                    WBd[bq * 32:(bq + 1) * 32, par * 64:(par + 1) * 64] = w2[dhB, d, :, :]
        wb[:, 128 + 128 * d:256 + 128 * d] = WAd
        wb[:, 512 + 128 * d:640 + 128 * d] = WBd

    # mask selector lhsTs (fp8): [128, 32*256]; pair i: cols 256i..256i+255,
    # tile0 at +0..127, tile1 (zeros) at +128..255.  mq partition p holds
    # mask row p-1 (p = 4q+bq = mask-slab row index).
    wm = np.zeros((128, 8192), np.float32)
    for t in range(16):
        m = np.zeros((128, 128), np.float32)
        m[4 * t + 1, 0:64] = LARGEM      # mask row 4t   -> out row 4t
        m[4 * t + 2, 64:128] = LARGEM    # mask row 4t+1 -> out row 4t+1
        wm[:, 256 * t:256 * t + 128] = m
        m = np.zeros((128, 128), np.float32)
        m[4 * t + 3, 0:64] = LARGEM      # mask row 4t+2 -> out row 4t+2
        m[4 * t + 4, 64:128] = LARGEM    # mask row 4t+3 -> out row 4t+3
        wm[:, 256 * (16 + t):256 * (16 + t) + 128] = m

    biases = np.zeros((128, 2), np.float32)
    biases[:, 0] = np.tile(b1, 4) - LARGE
    biases[:, 1] = np.tile(b2, 2) - LARGEM
    return (wb.ravel().astype(BF16), wm.ravel().astype(F8),
            biases.ravel().astype(np.float32))


def _prep_x(x, bn_gamma, bn_beta, bn_mean, bn_var):
    """Per-core DRAM slabs: pm (P + conv1 mask, bf16), mq (conv2 mask, fp8)."""
    s = float(bn_gamma[0] / np.sqrt(bn_var[0] + EPS))
    t = float(bn_beta[0] - bn_mean[0] * s)
    x = np.asarray(x, np.float32)[..., 0]        # [B,H,W]
    mask = (x != 0.0).astype(np.float32)
    P = s * x + t * mask
    # padded global arrays: rows -4..H+3 (pad 4 each side), cols pad 1
    Pg = np.zeros((B, H + 8, WP), np.float32)
    Mg = np.zeros((B, H + 8, WP), np.float32)
    Pg[:, 4:H + 4, 1:W + 1] = P
    Mg[:, 4:H + 4, 1:W + 1] = mask
    Pg16 = Pg.astype(BF16)
    Mg16 = Mg.astype(BF16)
    Mg8 = Mg[:, :, 1:W + 1].astype(F8)
    qrows = 4 * np.arange(NQ)
    pms, mqs = [], []
    for c in range(NCORES):
        r0 = c * ROWS
        pmc = np.empty((B, 22, NQ, 512), BF16)
        mqc = np.empty((B, MQB), F8)
        for b in range(B):
            Ps = Pg16[b, r0 + 2:r0 + 72]     # P rows r0-2 .. r0+67
            M1 = Mg16[b, r0 + 3:r0 + 71]     # mask rows r0-1 .. r0+66
            for s in range(6):
                for dwi in range(3):
                    pmc[b, s * 3 + dwi] = Ps[qrows + s][:, dwi:dwi + 512]
            for bq in range(4):
                pmc[b, 18 + bq] = M1[qrows + bq][:, 1:513]
            mqc[b] = Mg8[b, r0 + 3:r0 + 71].ravel()
        pms.append(pmc.ravel())
        mqs.append(mqc.ravel())
    return pms, mqs


def kernel(x, bn_gamma, bn_beta, bn_mean, bn_var, w1, b1, w2, b2):
    from concourse.bass_utils import run_bass_kernel_spmd

    if "nc" not in _cached:
        _cached["nc"] = _build_nc()
    nc = _cached["nc"]
    wb, wm, biases = _prep_consts(bn_gamma, bn_beta, bn_mean, bn_var,
                                  w1, b1, w2, b2)
    pms, mqs = _prep_x(x, bn_gamma, bn_beta, bn_mean, bn_var)
    in_maps = [{"pm": pms[c], "mq": mqs[c], "wb": wb, "wm": wm, "bias": biases}
               for c in range(NCORES)]
    res = run_bass_kernel_spmd(nc, in_maps, list(range(NCORES)))
    full = np.empty((B, 64, H, W), np.float32)
    for c in range(NCORES):
        # device layout [B][8 stage][2 par][64 co][4 pairsel][512]:
        # row = 8*stage + 2*pairsel + par
        o = (np.asarray(res.results[c]["out"]).astype(np.float32)
             .reshape(B, 8, 2, 64, 4, 512))
        o = o.transpose(0, 3, 1, 4, 2, 5).reshape(B, 64, ROWS, W)
        full[:, :, c * ROWS:(c + 1) * ROWS, :] = o
    return full


# revision 33
# speedup vs baseline: 1.0025x; 1.0025x over previous
"""Trainium2 Bass kernel for submanifold sparse conv net (gnn_message_passing).

Network: mask = (x != 0); y = BN(x) masked; y1 = relu(subm_conv3x3(y, w1) + b1);
y2 = relu(subm_conv3x3(y1, w2) + b2); out = NCHW(y2).  B,H,W = 4,512,512, C: 1->32->64.

Sharding: H split into 8 slabs of 64 rows (one per NeuronCore).

Per-core design ("quad" layout, channels*rows on partitions):

- conv1 computes FOUR y1 rows per matmul: PSUM [128 = 4 rows x 32 ch, 512].
  K=22 rhs rows are (row-shift s in 0..5) x (dw in -1..1) copies of the
  host-precombined plane P = bn_scale*x + bn_shift*mask (18 rows) plus 4
  shifted mask rows (submanifold restrict via +LARGE, bias -LARGE).
- y1 (bf16) is stored in two phase-interleaved banks of 4-row "slots" so
  every conv2 output row-pair (r, r+1) finds its full 4-row x 32-ch input
  window on 128 partitions in ONE slot: A-slots hold rows 4q-1..4q+2
  (pairs r=4t), B-slots hold rows 4w+1..4w+4 band-rotated (pairs r=4t+2).
  The B copies reuse the same conv1 PSUM partitions (no transpose).
- conv2: per row-pair, 3 bf16 matmuls [K=128, M=128 = 2 rows x 64 ch,
  N=512] (one per dw; dw = +-1 is a free-dim offset thanks to 1-col slot
  pads) plus ONE fp8 DoubleRow matmul adding +LARGEM*mask for both rows
  (mask rows live in a [128, 2, 516] fp8 tile, partition = slab row).
  Relu+bias+bf16-downcast splits across ACT/DVE; B-slot fills are DVE
  SBUF->SBUF copies; GPSIMD does memsets (it cannot access PSUM).
- Output written bf16 (upcast on host): halves the dominant DMA traffic.
"""

import sys

if "/opt/trn_rl_repo" not in sys.path:
    sys.path.insert(0, "/opt/trn_rl_repo")

import numpy as np
import ml_dtypes

BF16 = ml_dtypes.bfloat16
F8 = ml_dtypes.float8_e4m3   # device float8e4 has inf: max finite 240

B, H, W = 4, 512, 512
NCORES = 8
ROWS = H // NCORES          # 64 output rows per core
WP = 514                    # padded row width (1 zero col each side)
NQ = 17                     # conv1 quads per batch (y1 rows -1..66)
PR = 70                     # P slab rows (-2..67)
MR = 68                     # mask slab rows (-1..66)
PMB = 22 * NQ * 512         # pm elems per batch (pre-gathered conv1 rhs)
MQB = MR * 512              # mq elems per batch
ASLOTS = 17
BSLOTS = 16
BBASE = ASLOTS * WP         # B-slot region base inside y1 tile
REG = (ASLOTS + BSLOTS) * WP
OUTB = 64 * ROWS * W        # out elems per batch per core
LARGE = 256.0
LARGEM = 224.0  # mask weight in fp8e4m3 (256 would be inf)
EPS = 1e-5

_cached = {}


def _build_nc():
    import concourse.bass as bass
    import concourse.mybir as mybir
    from concourse import bacc, tile

    f32 = mybir.dt.float32
    bf16 = mybir.dt.bfloat16
    fp8 = mybir.dt.float8e4
    AP = bass.AP
    DR = mybir.MatmulPerfMode.DoubleRow
    Relu = mybir.ActivationFunctionType.Relu
    add_, max_ = mybir.AluOpType.add, mybir.AluOpType.max

    nc = bacc.Bacc("TRN2", target_bir_lowering=False, debug=False,
                   num_devices=NCORES)
    pm = nc.declare_dram_parameter("pm", [B * PMB], bf16, isOutput=False)
    mqd = nc.declare_dram_parameter("mq", [B * MQB], fp8, isOutput=False)
    wbd = nc.declare_dram_parameter("wb", [128 * 896], bf16, isOutput=False)
    wmd = nc.declare_dram_parameter("wm", [128 * 8192], fp8, isOutput=False)
    bid = nc.declare_dram_parameter("bias", [128 * 2], f32, isOutput=False)
    out = nc.declare_dram_parameter("out", [B * OUTB], bf16, isOutput=True)

    with tile.TileContext(nc) as tc:
        with (
            tc.tile_pool(name="const", bufs=1) as cpool,
            tc.tile_pool(name="rhs1", bufs=2) as rpool,
            tc.tile_pool(name="y1", bufs=2) as ypool,
            tc.tile_pool(name="mq", bufs=4) as mpool,
            tc.tile_pool(name="stage", bufs=3) as spool,
            tc.tile_pool(name="ps1", bufs=2, space="PSUM") as p1pool,
            tc.tile_pool(name="ps2", bufs=2, space="PSUM") as p2pool,
        ):
            wb = cpool.tile([128, 896], bf16, tag="wb")
            # wm as 3D [128, 64, 128]: pair i = dim1 slots 2i, 2i+1, so the
            # DR lhsT [K, 2, M] is a plain (tracked) slice
            wm = cpool.tile([128, 64, 128], fp8, tag="wm")
            bias = cpool.tile([128, 2], f32, tag="bias")
            nc.sync.dma_start(out=wb[:, :], in_=AP(wbd, 0, [[896, 128], [1, 896]]))
            nc.sync.dma_start(out=bias[:, :], in_=AP(bid, 0, [[2, 128], [1, 2]]))
            w1t = wb[0:22, 0:128]
            WA = [wb[0:128, 128 + 128 * d:256 + 128 * d] for d in range(3)]
            WBm = [wb[0:128, 512 + 128 * d:640 + 128 * d] for d in range(3)]
            bias1 = bias[:, 0:1]
            bias2 = bias[:, 1:2]
            # warm engines against the const DMA lanes so first real ops
            # don't exceed the per-instruction sync-wait limit
            scr = cpool.tile([128, 2], f32, tag="scr")
            nc.scalar.activation(scr[:, 0:1], bias[:, 0:1], Relu, bias=bias[:, 1:2])
            nc.vector.tensor_scalar(scr[:, 1:2], bias[:, 1:2], 0.0, 0.0, add_, max_)

            # GPSIMD cannot touch PSUM: relus (PSUM reads) go to ACT/DVE;
            # y1 B-slot fills are SBUF->SBUF bf16 copies (DVE 4x mode).
            def relu_op(eng, out_ap, in_ap, bias_ap):
                if eng == 0:
                    nc.scalar.activation(out_ap, in_ap, Relu, bias=bias_ap)
                else:
                    nc.vector.tensor_scalar(out_ap, in_ap, bias_ap, 0.0, add_, max_)

            y2rot = {"i": 0}
            # per 16-group batch: 7 on ACT, 9 on DVE
            Y2PAT = (1, 0, 1, 1, 0, 1, 0, 1, 1, 0, 1, 0, 1, 1, 0, 1)

            def next_y2():
                e = Y2PAT[y2rot["i"] % 16]
                y2rot["i"] += 1
                return e

            for b in range(B):
                rhs1 = rpool.tile([22, NQ * 512], bf16, tag="rhs1")
                nc.sync.dma_start(
                    out=rhs1[:, :],
                    in_=AP(pm, b * PMB, [[NQ * 512, 22], [1, NQ * 512]]))
                if b == 0:
                    # issue the big wm load after batch 0's input so the
                    # first conv1 isn't stuck behind its transfer
                    nc.sync.dma_start(out=wm[:, :, :],
                                      in_=AP(wmd, 0, [[8192, 128], [1, 8192]]))
                # mq 3D [128, 2, 516]: copy 0 holds mask rows (partition
                # 4q+bq = mask row 4q-1+bq), copy 1 stays zero -- the DR
                # rhs [:, 0:2, 0:512] is then a plain tracked slice (the
                # second tile multiplies zero weights anyway).
                mq = mpool.tile([128, 2, 516], fp8, tag="mq")
                nc.gpsimd.memset(mq[:, :, :], 0.0)
                nc.sync.dma_start(
                    out=mq[0:MR, 0:1, 0:512],
                    in_=AP(mqd, b * MQB, [[512, MR], [1, 512]]))
                # y1 as a 3D tile [128, 33 slots, 514]: slots 0..16 = A (rows
                # 4q-1..4q+2), 17..32 = B slot w (rows 4w+1..4w+4, bands
                # rotated).  All accesses are plain slices so the tile
                # scheduler tracks them (raw APs are invisible to it).
                y1 = ypool.tile([128, ASLOTS + BSLOTS, WP], bf16, tag="y1")
                nc.gpsimd.memset(y1[:, :, 0:1], 0.0)
                nc.gpsimd.memset(y1[:, :, WP - 1:WP], 0.0)

                stage_tiles = {}

                def emit_group(t):
                    ps2 = p2pool.tile([128, 1024], f32, tag="ps2")
                    # aligned pair (rows 4t, 4t+1) from A-slot t
                    for d in range(3):
                        nc.tensor.matmul(
                            ps2[:, 0:512], lhsT=WA[d],
                            rhs=y1[:, t:t + 1, d:d + 512],
                            start=(d == 0), stop=False)
                    nc.tensor.matmul(
                        ps2[:, 0:512],
                        lhsT=wm[:, 2 * t:2 * t + 2, 0:128],
                        rhs=mq[:, 0:2, 0:512],
                        start=False, stop=True, perf_mode=DR)
                    # split pair (rows 4t+2, 4t+3) from B-slot t
                    for d in range(3):
                        nc.tensor.matmul(
                            ps2[:, 512:1024], lhsT=WBm[d],
                            rhs=y1[:, ASLOTS + t:ASLOTS + t + 1, d:d + 512],
                            start=(d == 0), stop=False)
                    nc.tensor.matmul(
                        ps2[:, 512:1024],
                        lhsT=wm[:, 32 + 2 * t:32 + 2 * t + 2, 0:128],
                        rhs=mq[:, 0:2, 0:512],
                        start=False, stop=True, perf_mode=DR)
                    s = t // 2
                    if t % 2 == 0:
                        stage_tiles[s] = spool.tile([128, 2048], bf16,
                                                    tag="stage", name="stage")
                    st = stage_tiles[s]
                    relu_op(next_y2(), st[:, (t % 2) * 1024:(t % 2) * 1024 + 1024],
                            ps2[:, :], bias2)
                    if t % 2 == 1:
                        # out DRAM mirrors the stage layout exactly
                        # [B][8 stages][128 part][4 pairsel][512]; host fixes
                        # the ordering.  One 2-dim AP, 4KB descriptors.
                        nc.sync.dma_start(
                            out=AP(out, (b * 8 + s) * 128 * 2048,
                                   [[2048, 128], [1, 2048]]),
                            in_=st[:, :])
                        del stage_tiles[s]

                BA = ASLOTS  # B-slot w lives at y1 slot BA + w
                for k in range(9):
                    ps1 = p1pool.tile([128, 1024], f32, tag="ps1")
                    nhalf = 2 if k < 8 else 1
                    for h in range(nhalf):
                        q = 2 * k + h
                        nc.tensor.matmul(
                            ps1[:, 512 * h:512 * h + 512], lhsT=w1t,
                            rhs=rhs1[:, q * 512:q * 512 + 512],
                            start=True, stop=True)
                    if k < 8:
                        # A-write: rows 4(2k)-1 .. 4(2k+1)+2 -> A-slots 2k, 2k+1
                        nc.scalar.activation(
                            y1[:, 2 * k:2 * k + 2, 1:513],
                            ps1[:, :], Relu, bias=bias1)
                        # B-slot fills are SBUF->SBUF copies of A data (same
                        # partitions): B-low = bands 2,3 of A 2k, 2k+1;
                        # B-high = bands 0,1 of A 2k, 2k+1 -> B 2k-1, 2k.
                        nc.vector.tensor_scalar(
                            y1[64:128, BA + 2 * k:BA + 2 * k + 2, 1:513],
                            y1[64:128, 2 * k:2 * k + 2, 1:513],
                            0.0, None, add_)
                        if k == 0:
                            nc.vector.tensor_scalar(
                                y1[0:64, BA:BA + 1, 1:513],
                                y1[0:64, 1:2, 1:513],
                                0.0, None, add_)
                        else:
                            nc.vector.tensor_scalar(
                                y1[0:64, BA + 2 * k - 1:BA + 2 * k + 1, 1:513],
                                y1[0:64, 2 * k:2 * k + 2, 1:513],
                                0.0, None, add_)
                    else:
                        # lone quad 16: only bands 0,1 (rows 63, 64) -> B-slot 15
                        relu_op(0,
                                y1[0:64, BA + 15:BA + 16, 1:513],
                                ps1[0:64, 0:512], bias1[0:64, 0:1])
                    # conv2 groups, one k-iteration BEHIND readiness: the
                    # in-order PE queue then never waits on the ACT->DVE
                    # y1-fill chain (which also protects the PE p-state).
                    if k == 1:
                        emit_group(0)
                    elif 2 <= k <= 8:
                        emit_group(2 * k - 3)
                        emit_group(2 * k - 2)
                emit_group(15)
    nc.finalize()
    return nc


def _prep_consts(bn_gamma, bn_beta, bn_mean, bn_var, w1, b1, w2, b2):
    w1 = np.asarray(w1, np.float32)  # [3,3,1,32] (kh, kw, ci, co)
    w2 = np.asarray(w2, np.float32)  # [3,3,32,64]
    b1 = np.asarray(b1, np.float32)
    b2 = np.asarray(b2, np.float32)

    wb = np.zeros((128, 896), np.float32)
    # conv1 lhsT: rows kp = s*3+dwi (P copies), 18+b (mask rows); cols b*32+ch
    for s in range(6):
        for dwi in range(3):
            kp = s * 3 + dwi
            for bq in range(4):
                dh = s - bq
                if 0 <= dh <= 2:
                    wb[kp, 128 * 0:0] = 0  # noop for clarity
                    wb[kp, bq * 32:(bq + 1) * 32] = w1[dh, dwi, 0, :]
    for bq in range(4):
        wb[18 + bq, bq * 32:(bq + 1) * 32] = LARGE
    # conv2 aligned lhsT WA[d]: row (b*32+ch), col (par*64+co)
    for d in range(3):
        WAd = np.zeros((128, 128), np.float32)
        WBd = np.zeros((128, 128), np.float32)
        for bq in range(4):
            for par in range(2):
                dh = bq - par
                if 0 <= dh <= 2:
                    WAd[bq * 32:(bq + 1) * 32, par * 64:(par + 1) * 64] = w2[dh, d, :, :]
                dhB = (2 - par, 3 - par, -par, 1 - par)[bq]
                if 0 <= dhB <= 2:
                    WBd[bq * 32:(bq + 1) * 32, par * 64:(par + 1) * 64] = w2[dhB, d, :, :]
        wb[:, 128 + 128 * d:256 + 128 * d] = WAd
        wb[:, 512 + 128 * d:640 + 128 * d] = WBd

    # mask selector lhsTs (fp8): [128, 32*256]; pair i: cols 256i..256i+255,
    # tile0 at +0..127, tile1 (zeros) at +128..255.  mq partition p holds
    # mask row p-1 (p = 4q+bq = mask-slab row index).
    wm = np.zeros((128, 8192), np.float32)
    for t in range(16):
        m = np.zeros((128, 128), np.float32)
        m[4 * t + 1, 0:64] = LARGEM      # mask row 4t   -> out row 4t
        m[4 * t + 2, 64:128] = LARGEM    # mask row 4t+1 -> out row 4t+1
        wm[:, 256 * t:256 * t + 128] = m
        m = np.zeros((128, 128), np.float32)
        m[4 * t + 3, 0:64] = LARGEM      # mask row 4t+2 -> out row 4t+2
        m[4 * t + 4, 64:128] = LARGEM    # mask row 4t+3 -> out row 4t+3
        wm[:, 256 * (16 + t):256 * (16 + t) + 128] = m

    biases = np.zeros((128, 2), np.float32)
    biases[:, 0] = np.tile(b1, 4) - LARGE
    biases[:, 1] = np.tile(b2, 2) - LARGEM
    return (wb.ravel().astype(BF16), wm.ravel().astype(F8),
            biases.ravel().astype(np.float32))


def _prep_x(x, bn_gamma, bn_beta, bn_mean, bn_var):
    """Per-core DRAM slabs: pm (P + conv1 mask, bf16), mq (conv2 mask, fp8)."""
    s = float(bn_gamma[0] / np.sqrt(bn_var[0] + EPS))
    t = float(bn_beta[0] - bn_mean[0] * s)
    x = np.asarray(x, np.float32)[..., 0]        # [B,H,W]
    mask = (x != 0.0).astype(np.float32)
    P = s * x + t * mask
    # padded global arrays: rows -4..H+3 (pad 4 each side), cols pad 1
    Pg = np.zeros((B, H + 8, WP), np.float32)
    Mg = np.zeros((B, H + 8, WP), np.float32)
    Pg[:, 4:H + 4, 1:W + 1] = P
    Mg[:, 4:H + 4, 1:W + 1] = mask
    Pg16 = Pg.astype(BF16)
    Mg16 = Mg.astype(BF16)
    Mg8 = Mg[:, :, 1:W + 1].astype(F8)
    qrows = 4 * np.arange(NQ)
    pms, mqs = [], []
    for c in range(NCORES):
        r0 = c * ROWS
        pmc = np.empty((B, 22, NQ, 512), BF16)
        mqc = np.empty((B, MQB), F8)
        for b in range(B):
            Ps = Pg16[b, r0 + 2:r0 + 72]     # P rows r0-2 .. r0+67
            M1 = Mg16[b, r0 + 3:r0 + 71]     # mask rows r0-1 .. r0+66
            for s in range(6):
                for dwi in range(3):
                    pmc[b, s * 3 + dwi] = Ps[qrows + s][:, dwi:dwi + 512]
            for bq in range(4):
                pmc[b, 18 + bq] = M1[qrows + bq][:, 1:513]
            mqc[b] = Mg8[b, r0 + 3:r0 + 71].ravel()
        pms.append(pmc.ravel())
        mqs.append(mqc.ravel())
    return pms, mqs


def kernel(x, bn_gamma, bn_beta, bn_mean, bn_var, w1, b1, w2, b2):
    from concourse.bass_utils import run_bass_kernel_spmd

    if "nc" not in _cached:
        _cached["nc"] = _build_nc()
    nc = _cached["nc"]
    wb, wm, biases = _prep_consts(bn_gamma, bn_beta, bn_mean, bn_var,
                                  w1, b1, w2, b2)
    pms, mqs = _prep_x(x, bn_gamma, bn_beta, bn_mean, bn_var)
    in_maps = [{"pm": pms[c], "mq": mqs[c], "wb": wb, "wm": wm, "bias": biases}
               for c in range(NCORES)]
    res = run_bass_kernel_spmd(nc, in_maps, list(range(NCORES)))
    full = np.empty((B, 64, H, W), np.float32)
    for c in range(NCORES):
        # device layout [B][8 stage][2 par][64 co][4 pairsel][512]:
        # row = 8*stage + 2*pairsel + par
        o = (np.asarray(res.results[c]["out"]).astype(np.float32)
             .reshape(B, 8, 2, 64, 4, 512))
        o = o.transpose(0, 3, 1, 4, 2, 5).reshape(B, 64, ROWS, W)
        full[:, :, c * ROWS:(c + 1) * ROWS, :] = o
    return full
